# revision 1
# baseline (speedup 1.0000x reference)
"""ConvLSTM2D block (ConvLSTM -> BatchNorm -> MaxPool2x2) on 8 Trainium2 cores.

Problem (hardcoded): x [B=4, T=16, H=64, W=64, Cin=64], ConvLSTM2D with
3x3 kernels, C=64 channels, keras gate order (i, f, g, o), hard_sigmoid
recurrent activation, tanh activation, inference BatchNorm, spatial 2x2
max pool -> out [4, 16, 32, 32, 64] fp32.

Sharding: 8 shards = batch(4) x H-halves(2). Each core computes a 48-row
slice (rows 0..47 top half, 16..63 bottom) of one sample's full
recurrence. The 16-row overlap is recomputed redundantly: a 3x3 recurrent
conv corrupts one boundary row per timestep, so after 16 steps the 32
rows each core owns are still exact. No cross-core communication.

Per core, per timestep: z = conv(x_t, W) + conv(h_t, U) via f32r matmuls
into PSUM. Channels sit on partitions; each channel's spatial plane is a
padded [50, 66] row-major strip, x_t's plane in partitions 0:63 and h_t's
in 64:127, so each of the 9 taps is ONE K=128 matmul whose stationary
operand stacks [W_tap; U_tap] - input and recurrent convs fuse for free.
Cout is reordered (f, i, o, g) so the two M=128 halves give PSUM tiles
[f;i] and [o;g] and the LSTM pointwise update is partition-aligned except
one 64-partition SBUF->SBUF DMA (i*g product move) per 512-pixel block.
"""
import sys
sys.path.insert(0, '/opt/trn_rl_repo')

import numpy as np

import bass_rust
import concourse.bass as bass
import concourse.tile as tile
from concourse import mybir
from concourse.bass_utils import run_bass_kernel_spmd

F32 = mybir.dt.float32
F32R = mybir.dt.float32r
ALU = mybir.AluOpType
ACTF = mybir.ActivationFunctionType

B, T, H, W, C = 4, 16, 64, 64, 64
BN_EPS = 1e-3
HS = 48           # rows per shard
RP, CP = 50, 66   # padded plane rows/cols
NBLK = 6          # pixel blocks per step: 8 data rows x 64 cols = 512 px

_cached = None
_REPS = None   # bench-only: on-device repeat of the whole timestep loop
_MM_ONLY = False  # bench-only: emit just the matmuls
_NO_PRM = False   # bench-only: skip the cross-partition product move


def _split_multi_waits(nc, limit=1):
    """walrus here encodes at most one sem-wait per instruction; move excess
    waits onto nops inserted before the instruction on the same engine."""
    cnt = 0
    for fn in nc.m.functions:
        for bb in fn.blocks:
            out, changed = [], False
            for inst in bb.instructions:
                si = inst.sync_info
                waits = list(si.on_wait) if (si and si.on_wait) else []
                if len(waits) > limit:
                    changed = True
                    extra, keep = waits[:-limit], waits[-limit:]
                    for i in range(0, len(extra), limit):
                        cnt += 1
                        nop = mybir.InstNoOp(name=f"I-wsplit-{cnt}", engine=inst.engine)
                        nop.sync_info = bass_rust.SyncInfo(
                            on_wait=extra[i:i + limit], on_update=[])
                        out.append(nop)
                    si.on_wait = keep
                out.append(inst)
            if changed:
                bb.instructions = out


def _build():
    nc = bass.Bass()
    x_d = nc.dram_tensor("xc", [T, C, HS, W], F32R, kind="ExternalInput")
    w_d = nc.dram_tensor("wstk", [128, 9, 256], F32R, kind="ExternalInput")
    cn_d = nc.dram_tensor("consts", [128, 4], F32, kind="ExternalInput")
    z_d = nc.dram_tensor("zeros", [128, RP * CP], F32R, kind="ExternalInput")
    y_d = nc.dram_tensor("yout", [T, C, 24 * 32], F32, kind="ExternalOutput")

    with tile.TileContext(nc) as tc:
        with (
            tc.tile_pool(name="state", bufs=1) as st,
            tc.tile_pool(name="scr", bufs=6) as sc,
            tc.tile_pool(name="pool_scr", bufs=2) as pscr,
            tc.tile_pool(name="psum", bufs=4, space="PSUM") as pp,
        ):
            wsb = st.tile([128, 9, 256], F32R, tag="wsb")
            nc.sync.dma_start(out=wsb, in_=w_d[:, :, :])
            cons = st.tile([128, 4], F32, tag="cons")
            nc.sync.dma_start(out=cons, in_=cn_d[:, :])
            bvfi = cons[:, 0:1]
            bvo = cons[0:64, 1:2]
            bg = cons[64:128, 1:2]
            bns = cons[0:64, 2:3]
            bnb = cons[0:64, 3:4]

            # xh planes: partitions 0:63 = x_t, 64:127 = h_t, double buffered
            xh = [st.tile([128, RP * CP], F32R, tag=f"xh{i}", name=f"xh{i}")
                  for i in range(2)]
            cg = st.tile([128, HS * W], F32, tag="cg")
            for tns in xh:
                nc.sync.dma_start(out=tns, in_=z_d[:, :])
            nc.vector.memset(cg, 0.0)

            def pv(tns):
                return tns.rearrange("p (r c) -> p r c", r=RP)

            pv0 = pv(xh[0])
            nc.sync.dma_start(out=pv0[0:64, 1:49, 1:65], in_=x_d[0, :, :, :])

            import contextlib
            rep_cm = tc.For_i(0, _REPS, 1) if _REPS else contextlib.nullcontext()
            with rep_cm:
              for t in range(T):
                  cur = pv(xh[t % 2])
                  nxt = pv(xh[(t + 1) % 2])
                  if _REPS or t + 1 < T:
                      nc.sync.dma_start(out=nxt[0:64, 1:49, 1:65],
                                        in_=x_d[(t + 1) % T, :, :, :])
                  hpool = sc.tile([64, HS * W], F32R, tag="hpool",
                                  name=f"hpool{t}")
                  hpv = hpool.rearrange("p (r c) -> p r c", r=HS)

                  for blk in range(NBLK):
                      r0 = blk * 8
                      fs = slice(blk * 512, (blk + 1) * 512)
                      pst = []
                      for mh in range(2):
                          ps = pp.tile([128, 512], F32, tag=f"ps{mh}",
                                       name=f"ps_{t}_{blk}_{mh}")
                          pst.append(ps)
                          for j in range(9):
                              a0, b0 = j // 3, j % 3
                              rhs = cur[:, r0 + a0:r0 + a0 + 8, b0:b0 + 64]
                              nc.tensor.matmul(
                                  ps, wsb[:, j, mh * 128:(mh + 1) * 128], rhs,
                                  start=(j == 0), stop=(j == 8))

                      if _MM_ONLY:
                          continue
                      ps0, ps1 = pst
                      # f,i: hard_sigmoid = clip(0.2 z + (0.2 b + .5), 0, 1) on DVE
                      if2 = sc.tile([128, 512], F32, tag="if2")
                      nc.vector.tensor_scalar(if2, ps0, 0.2, bvfi, ALU.mult, ALU.add)
                      nc.vector.tensor_scalar(if2, if2, 0.0, 1.0, ALU.max, ALU.min)
                      # o: affine on ACT (Identity shares the tanh table set),
                      # clip on gpsimd
                      oo = sc.tile([64, 512], F32, tag="oo")
                      nc.scalar.activation(oo, ps1[0:64, :], ACTF.Identity,
                                           bias=bvo, scale=0.2)
                      nc.gpsimd.tensor_scalar(oo, oo, 0.0, 1.0, ALU.max, ALU.min)
                      # g -> cg[64:128]
                      nc.scalar.activation(cg[64:128, fs], ps1[64:128, :],
                                           ACTF.Tanh, bias=bg, scale=1.0)
                      # pr = [f*c ; i*g]
                      pr = sc.tile([128, 512], F32, tag="pr")
                      nc.vector.tensor_tensor(pr, if2, cg[:, fs], ALU.mult)
                      if _NO_PRM:
                          nc.vector.tensor_tensor(cg[0:64, fs], pr[0:64, :],
                                                  pr[0:64, :], ALU.add)
                      else:
                          prm = sc.tile([64, 512], F32, tag="prm")
                          nc.sync.dma_start(out=prm, in_=pr[64:128, :])
                          nc.vector.tensor_tensor(cg[0:64, fs], pr[0:64, :], prm,
                                                  ALU.add)
                      # h = o * tanh(c)
                      tct = sc.tile([64, 512], F32, tag="tct")
                      nc.scalar.activation(tct, cg[0:64, fs], ACTF.Tanh)
                      nc.vector.tensor_tensor(hpool[:, fs], oo, tct, ALU.mult)
                      # h block -> next plane's upper half
                      nc.sync.dma_start(out=nxt[64:128, r0 + 1:r0 + 9, 1:65],
                                        in_=hpv[:, r0:r0 + 8, :])

                  if _MM_ONLY:
                      continue
                  # BN + 2x2 max pool on h_{t+1}
                  s1 = pscr.tile([64, 48, 32], F32, tag="s1")
                  nc.vector.tensor_tensor(s1, hpv[:, :, 0:64:2], hpv[:, :, 1:64:2],
                                          ALU.max)
                  s2 = pscr.tile([64, 24, 32], F32, tag="s2")
                  nc.vector.tensor_tensor(s2, s1[:, 0:48:2, :], s1[:, 1:48:2, :],
                                          ALU.max)
                  yt = pscr.tile([64, 24 * 32], F32, tag="yt")
                  nc.vector.tensor_scalar(yt, s2.rearrange("p a b -> p (a b)"),
                                          bns, bnb, ALU.mult, ALU.add)
                  nc.sync.dma_start(out=y_d[t, :, :], in_=yt)

    _split_multi_waits(nc)
    return nc


def _get_nc():
    global _cached
    if _cached is None:
        _cached = _build()
    return _cached


def kernel(input_tensor, W, U, b, gamma, beta, moving_mean, moving_var):
    x = np.asarray(input_tensor, np.float32)
    W = np.asarray(W, np.float32)
    U = np.asarray(U, np.float32)
    b = np.asarray(b, np.float32)
    gamma = np.asarray(gamma, np.float32)
    beta = np.asarray(beta, np.float32)
    moving_mean = np.asarray(moving_mean, np.float32)
    moving_var = np.asarray(moving_var, np.float32)

    # Cout reorder (i,f,g,o) -> (f,i,o,g)
    perm = [1, 0, 3, 2]
    Wr = W.reshape(3, 3, C, 4, C)[:, :, :, perm, :].reshape(3, 3, C, 4 * C)
    Ur = U.reshape(3, 3, C, 4, C)[:, :, :, perm, :].reshape(3, 3, C, 4 * C)
    # stationary operands: tap j rows 0:64 = W tap (x half of the plane),
    # rows 64:128 = U tap (h half)
    wstk = np.zeros((9, 128, 256), np.float32)
    for j in range(9):
        a0, b0 = j // 3, j % 3
        wstk[j, 0:64] = Wr[a0, b0]
        wstk[j, 64:128] = Ur[a0, b0]
    wstk = np.ascontiguousarray(wstk.transpose(1, 0, 2))  # [128, 9, 256]

    b4 = b.reshape(4, C)[perm]  # rows f,i,o,g
    consts = np.zeros((128, 4), np.float32)
    consts[0:64, 0] = 0.2 * b4[0] + 0.5
    consts[64:128, 0] = 0.2 * b4[1] + 0.5
    consts[0:64, 1] = 0.2 * b4[2] + 0.5
    consts[64:128, 1] = b4[3]
    scale = gamma / np.sqrt(moving_var + BN_EPS)
    consts[0:64, 2] = scale
    consts[0:64, 3] = beta - moving_mean * scale

    zplane = np.zeros((128, RP * CP), np.float32)
    in_maps = []
    for k in range(8):
        s, half = k // 2, k % 2
        r0 = 0 if half == 0 else H - HS
        xs = np.ascontiguousarray(x[s, :, r0:r0 + HS].transpose(0, 3, 1, 2))
        in_maps.append({"xc": xs, "wstk": wstk, "consts": consts,
                        "zeros": zplane})

    res = run_bass_kernel_spmd(_get_nc(), in_maps, core_ids=list(range(8)))

    out = np.empty((B, T, 32, 32, C), np.float32)
    for k in range(8):
        s, half = k // 2, k % 2
        yc = res.results[k]["yout"].reshape(T, C, 24, 32).transpose(0, 2, 3, 1)
        if half == 0:
            out[s, :, 0:16] = yc[:, 0:16]
        else:
            out[s, :, 16:32] = yc[:, 8:24]
    return out



# revision 2
# speedup vs baseline: 2.0709x; 2.0709x over previous
"""ConvLSTM2D block (ConvLSTM -> BatchNorm -> MaxPool2x2) on 8 Trainium2 cores.

Problem (hardcoded): x [B=4, T=16, H=64, W=64, Cin=64], ConvLSTM2D with
3x3 kernels, C=64 channels, keras gate order (i, f, g, o), hard_sigmoid
recurrent activation, tanh activation, inference BatchNorm, spatial 2x2
max pool -> out [4, 16, 32, 32, 64] fp32.

Sharding: 8 shards = batch(4) x H-halves(2). Each core computes a 48-row
slice (rows 0..47 top half, 16..63 bottom) of one sample's full
recurrence. The 16-row overlap is recomputed redundantly: a 3x3 recurrent
conv corrupts one boundary row per timestep, so after 16 steps the 32
rows each core owns are still exact. No cross-core communication.

v2 layout (all activations bf16; PSUM accumulate fp32):
- plane [128, 50, 66] bf16: partitions 0:64 = h_t (so the LSTM pointwise,
  whose results land on partitions 0:64, writes h straight into the next
  plane with a strided DVE op - no SBUF->SBUF h copy), 64:128 = x_t.
- stationary tap j: rows 0:64 = U tap, 64:128 = W tap; gate columns
  [f,i | o,g]; f,i,o columns pre-scaled by +-0.2 so hard_sigmoid becomes
  bias+clip only. Two M=128 PSUM tiles per 512-px block: [f;i], [o;g].
- matmuls run in block PAIRS sharing each stationary tile (2 matmuls per
  weight load), 4 PSUM tiles/pair x 2 pairs in flight = 8 banks.
- pointwise: DVE does fi-clip (2 ops), products and the c update; ACT does
  the o hard_sigmoid as a Relu chain, both tanhs, and the BN affine; the
  only cross-partition move is the i*g fold (one 64-part DMA per block).
"""
import sys
sys.path.insert(0, '/opt/trn_rl_repo')

import numpy as np
import ml_dtypes

import bass_rust
import concourse.bass as bass
import concourse.tile as tile
from concourse import mybir
from concourse.bass_utils import run_bass_kernel_spmd

F32 = mybir.dt.float32
BF16 = mybir.dt.bfloat16
ALU = mybir.AluOpType
ACTF = mybir.ActivationFunctionType

B, T, H, W, C = 4, 16, 64, 64, 64
BN_EPS = 1e-3
HS = 48           # rows per shard
RP, CP = 50, 66   # padded plane rows/cols
NBLK = 6          # pixel blocks per step: 8 data rows x 64 cols = 512 px

_cached = None


def _split_multi_waits(nc, limit=1):
    """walrus here encodes at most one sem-wait per instruction; move excess
    waits onto nops inserted before the instruction on the same engine."""
    cnt = 0
    for fn in nc.m.functions:
        for bb in fn.blocks:
            out, changed = [], False
            for inst in bb.instructions:
                si = inst.sync_info
                waits = list(si.on_wait) if (si and si.on_wait) else []
                if len(waits) > limit:
                    changed = True
                    extra, keep = waits[:-limit], waits[-limit:]
                    for i in range(0, len(extra), limit):
                        cnt += 1
                        nop = mybir.InstNoOp(name=f"I-wsplit-{cnt}", engine=inst.engine)
                        nop.sync_info = bass_rust.SyncInfo(
                            on_wait=extra[i:i + limit], on_update=[])
                        out.append(nop)
                    si.on_wait = keep
                out.append(inst)
            if changed:
                bb.instructions = out


def _build():
    nc = bass.Bass()
    x_d = nc.dram_tensor("xc", [T, C, HS, W], BF16, kind="ExternalInput")
    w_d = nc.dram_tensor("wstk", [128, 9, 256], BF16, kind="ExternalInput")
    cn_d = nc.dram_tensor("consts", [128, 4], F32, kind="ExternalInput")
    y_d = nc.dram_tensor("yout", [T, C, 24 * 32], F32, kind="ExternalOutput")

    with tile.TileContext(nc) as tc:
        with (
            tc.tile_pool(name="state", bufs=1) as st,
            tc.tile_pool(name="scr", bufs=3) as sc,
            tc.tile_pool(name="pool_scr", bufs=2) as pscr,
            tc.tile_pool(name="psum", bufs=2, space="PSUM") as pp,
        ):
            wsb = st.tile([128, 9, 256], BF16, tag="wsb")
            nc.sync.dma_start(out=wsb, in_=w_d[:, :, :])
            cons = st.tile([128, 4], F32, tag="cons")
            nc.sync.dma_start(out=cons, in_=cn_d[:, :])
            b_fi = cons[:, 0:1]       # 0.2*b_f + 0.5 ; 0.2*b_i + 0.5
            b_o = cons[0:64, 1:2]     # 0.5 - 0.2*b_o
            b_g = cons[64:128, 1:2]   # b_g
            bns = cons[0:64, 2:3]     # BN scale
            bnb = cons[0:64, 3:4]     # BN bias

            # planes: partitions 0:63 = h_t, 64:127 = x_t, double buffered
            xh = [st.tile([128, RP * CP], BF16, tag=f"xh{i}", name=f"xh{i}")
                  for i in range(2)]
            cg = st.tile([128, HS * W], BF16, tag="cg")
            nc.gpsimd.memset(xh[0], 0.0)
            nc.gpsimd.memset(xh[1], 0.0)
            nc.vector.memset(cg, 0.0)

            def pv(tns):
                return tns.rearrange("p (r c) -> p r c", r=RP)

            nc.sync.dma_start(out=pv(xh[0])[64:128, 1:49, 1:65],
                              in_=x_d[0, :, :, :])

            for t in range(T):
                cur = pv(xh[t % 2])
                nxt = pv(xh[(t + 1) % 2])
                if t + 1 < T:
                    nc.sync.dma_start(out=nxt[64:128, 1:49, 1:65],
                                      in_=x_d[t + 1, :, :, :])

                for pair in range(NBLK // 2):
                    blks = (2 * pair, 2 * pair + 1)
                    ps = {}
                    for mh in range(2):
                        for b01 in range(2):
                            ps[mh, b01] = pp.tile(
                                [128, 512], F32, tag=f"ps{mh}{b01}",
                                name=f"ps_{t}_{pair}_{mh}{b01}")
                    # two matmuls (the pair's two blocks) per stationary tile
                    for mh in range(2):
                        for j in range(9):
                            a0, b0 = j // 3, j % 3
                            for b01, blk in enumerate(blks):
                                r0 = blk * 8
                                rhs = cur[:, r0 + a0:r0 + a0 + 8, b0:b0 + 64]
                                nc.tensor.matmul(
                                    ps[mh, b01],
                                    wsb[:, j, mh * 128:(mh + 1) * 128], rhs,
                                    start=(j == 0), stop=(j == 8))

                    for b01, blk in enumerate(blks):
                        r0 = blk * 8
                        fs = slice(blk * 512, (blk + 1) * 512)
                        ps0, ps1 = ps[0, b01], ps[1, b01]
                        # f,i: clip(z' + b', 0, 1) on DVE (scale folded in W)
                        fi2 = sc.tile([128, 512], BF16, tag="fi2")
                        nc.vector.tensor_scalar(fi2, ps0, b_fi, 0.0,
                                                ALU.add, ALU.max)
                        nc.vector.tensor_scalar_min(fi2, fi2, 1.0)
                        # o: hard_sigmoid via Relu chain on ACT
                        # (o columns folded with -0.2): relu(1 - relu(z''+b''))
                        oo1 = sc.tile([64, 512], BF16, tag="oo1")
                        nc.scalar.activation(oo1, ps1[0:64, :], ACTF.Relu,
                                             bias=b_o, scale=1.0)
                        oo = sc.tile([64, 512], BF16, tag="oo")
                        nc.scalar.activation(oo, oo1, ACTF.Relu,
                                             bias=1.0, scale=-1.0)
                        # g -> cg[64:128]
                        nc.scalar.activation(cg[64:128, fs], ps1[64:128, :],
                                             ACTF.Tanh, bias=b_g, scale=1.0)
                        # pr = [f*c ; i*g]
                        pr = sc.tile([128, 512], BF16, tag="pr")
                        nc.vector.tensor_tensor(pr, fi2, cg[:, fs], ALU.mult)
                        prm = sc.tile([64, 512], BF16, tag="prm")
                        nc.sync.dma_start(out=prm, in_=pr[64:128, :])
                        nc.vector.tensor_tensor(cg[0:64, fs], pr[0:64, :], prm,
                                                ALU.add)
                        # h = o * tanh(c), written straight into next plane
                        tct = sc.tile([64, 512], BF16, tag="tct")
                        nc.scalar.activation(tct, cg[0:64, fs], ACTF.Tanh)
                        nc.vector.tensor_tensor(
                            nxt[0:64, r0 + 1:r0 + 9, 1:65],
                            tct.rearrange("p (r c) -> p r c", r=8),
                            oo.rearrange("p (r c) -> p r c", r=8), ALU.mult)

                # BN + 2x2 max pool on h_{t+1} (read from the plane)
                s1 = pscr.tile([64, 48, 32], BF16, tag="s1")
                nc.vector.tensor_tensor(s1, nxt[0:64, 1:49, 1:65:2],
                                        nxt[0:64, 1:49, 2:66:2], ALU.max)
                s2 = pscr.tile([64, 24, 32], BF16, tag="s2")
                nc.vector.tensor_tensor(s2, s1[:, 0:48:2, :], s1[:, 1:48:2, :],
                                        ALU.max)
                yt = pscr.tile([64, 24 * 32], F32, tag="yt")
                nc.scalar.activation(yt, s2.rearrange("p a b -> p (a b)"),
                                     ACTF.Identity, bias=bnb, scale=bns)
                nc.sync.dma_start(out=y_d[t, :, :], in_=yt)

    _split_multi_waits(nc)
    return nc


def _get_nc():
    global _cached
    if _cached is None:
        _cached = _build()
    return _cached


def _prep_inputs(input_tensor, W_, U, b, gamma, beta, moving_mean, moving_var):
    x = np.asarray(input_tensor, np.float32)
    W_ = np.asarray(W_, np.float32)
    U = np.asarray(U, np.float32)
    b = np.asarray(b, np.float32)
    gamma = np.asarray(gamma, np.float32)
    beta = np.asarray(beta, np.float32)
    moving_mean = np.asarray(moving_mean, np.float32)
    moving_var = np.asarray(moving_var, np.float32)

    # Cout reorder (i,f,g,o) -> (f,i,o,g); fold hard_sigmoid scales into
    # the f,i (+0.2) and o (-0.2) columns.
    perm = [1, 0, 3, 2]
    gs = np.array([0.2, 0.2, -0.2, 1.0], np.float32)[None, None, None, :, None]
    Wr = (W_.reshape(3, 3, C, 4, C)[:, :, :, perm, :] * gs).reshape(3, 3, C, 4 * C)
    Ur = (U.reshape(3, 3, C, 4, C)[:, :, :, perm, :] * gs).reshape(3, 3, C, 4 * C)
    # stationary tap j: rows 0:64 = U tap (h half), rows 64:128 = W tap (x)
    wstk = np.zeros((9, 128, 256), np.float32)
    for j in range(9):
        a0, b0 = j // 3, j % 3
        wstk[j, 0:64] = Ur[a0, b0]
        wstk[j, 64:128] = Wr[a0, b0]
    wstk = np.ascontiguousarray(wstk.transpose(1, 0, 2)).astype(
        ml_dtypes.bfloat16)  # [128, 9, 256]

    b4 = b.reshape(4, C)[perm]  # rows f,i,o,g
    consts = np.zeros((128, 4), np.float32)
    consts[0:64, 0] = 0.2 * b4[0] + 0.5
    consts[64:128, 0] = 0.2 * b4[1] + 0.5
    consts[0:64, 1] = 0.5 - 0.2 * b4[2]
    consts[64:128, 1] = b4[3]
    scale = gamma / np.sqrt(moving_var + BN_EPS)
    consts[0:64, 2] = scale
    consts[0:64, 3] = beta - moving_mean * scale

    in_maps = []
    for k in range(8):
        s, half = k // 2, k % 2
        r0 = 0 if half == 0 else H - HS
        xs = np.ascontiguousarray(
            x[s, :, r0:r0 + HS].transpose(0, 3, 1, 2)).astype(
                ml_dtypes.bfloat16)
        in_maps.append({"xc": xs, "wstk": wstk, "consts": consts})
    return in_maps


def kernel(input_tensor, W, U, b, gamma, beta, moving_mean, moving_var):
    in_maps = _prep_inputs(input_tensor, W, U, b, gamma, beta,
                           moving_mean, moving_var)
    res = run_bass_kernel_spmd(_get_nc(), in_maps, core_ids=list(range(8)))

    out = np.empty((B, T, 32, 32, C), np.float32)
    for k in range(8):
        s, half = k // 2, k % 2
        yc = res.results[k]["yout"].reshape(T, C, 24, 32).transpose(0, 2, 3, 1)
        if half == 0:
            out[s, :, 0:16] = yc[:, 0:16]
        else:
            out[s, :, 16:32] = yc[:, 8:24]
    return out


# revision 3
# speedup vs baseline: 2.1323x; 1.0296x over previous
"""ConvLSTM2D block (ConvLSTM -> BatchNorm -> MaxPool2x2) on 8 Trainium2 cores.

Problem (hardcoded): x [B=4, T=16, H=64, W=64, Cin=64], ConvLSTM2D with
3x3 kernels, C=64 channels, keras gate order (i, f, g, o), hard_sigmoid
recurrent activation, tanh activation, inference BatchNorm, spatial 2x2
max pool -> out [4, 16, 32, 32, 64] fp32.

Sharding: 8 shards = batch(4) x H-halves(2). Each core computes a 48-row
slice of one sample's recurrence; the 16-row overlap is recomputed
redundantly (a 3x3 recurrent conv corrupts one boundary row per step).
Bottom-half shards are fed ROW-FLIPPED data (and row-flipped conv taps)
so that every core's owned rows are local rows 0:32 - this makes the
per-step compute window shrinkable in the same SPMD program: step t only
needs rows 0:48-t correct, so t>=8 runs 5 blocks instead of 6, and the
pool/BN/store stage covers only rows 0:32.

Layout (activations bf16, PSUM accumulate fp32):
- plane [128, 50, 66] bf16: partitions 0:64 = h_t (the LSTM pointwise
  lands on partitions 0:64, writing h straight into the next plane with a
  strided DVE op), 64:128 = x_t.
- stationary tap j: rows 0:64 = U tap, 64:128 = W tap; gate columns
  [f,i | o,g]; f,i (o) columns pre-scaled by 0.2 (-0.2) so hard_sigmoid
  needs no multiply. Per 512-px block two M=128 PSUM groups: [f;i], [o;g].
- pointwise: DVE clips f,i and forms the products/c update; ACT does the
  g and c tanhs plus the o hard_sigmoid as a Relu chain (g first - it is
  on the h critical path); the only cross-partition move is the i*g fold
  (one 64-part DMA per block, issued on the Pool engine's DGE).
- a few warmup matmuls at t=0 ramp the PE p-state while x DMAs land.
"""
import sys
sys.path.insert(0, '/opt/trn_rl_repo')

import numpy as np
import ml_dtypes

import bass_rust
import concourse.bass as bass
import concourse.tile as tile
from concourse import mybir
from concourse.bass_utils import run_bass_kernel_spmd

F32 = mybir.dt.float32
BF16 = mybir.dt.bfloat16
ALU = mybir.AluOpType
ACTF = mybir.ActivationFunctionType

B, T, H, W, C = 4, 16, 64, 64, 64
BN_EPS = 1e-3
HS = 48           # rows per shard
RP, CP = 50, 66   # padded plane rows/cols

_cached = None


def _split_multi_waits(nc, limit=1):
    """walrus here encodes at most one sem-wait per instruction; move excess
    waits onto nops inserted before the instruction on the same engine."""
    cnt = 0
    for fn in nc.m.functions:
        for bb in fn.blocks:
            out, changed = [], False
            for inst in bb.instructions:
                si = inst.sync_info
                waits = list(si.on_wait) if (si and si.on_wait) else []
                if len(waits) > limit:
                    changed = True
                    extra, keep = waits[:-limit], waits[-limit:]
                    for i in range(0, len(extra), limit):
                        cnt += 1
                        nop = mybir.InstNoOp(name=f"I-wsplit-{cnt}", engine=inst.engine)
                        nop.sync_info = bass_rust.SyncInfo(
                            on_wait=extra[i:i + limit], on_update=[])
                        out.append(nop)
                    si.on_wait = keep
                out.append(inst)
            if changed:
                bb.instructions = out


def _nblk(t):
    # step t needs h_{t+1} rows 0:48-t correct -> ceil((48-t)/8) blocks
    return -(-(48 - t) // 8)


def _build():
    nc = bass.Bass()
    x_d = nc.dram_tensor("xc", [T, C, HS, W], BF16, kind="ExternalInput")
    w_d = nc.dram_tensor("wstk", [128, 9, 256], BF16, kind="ExternalInput")
    cn_d = nc.dram_tensor("consts", [128, 4], F32, kind="ExternalInput")
    z_d = nc.dram_tensor("zeros", [128, RP * CP], BF16, kind="ExternalInput")
    y_d = nc.dram_tensor("yout", [T, C, 16 * 32], F32, kind="ExternalOutput")

    with tile.TileContext(nc) as tc:
        with (
            tc.tile_pool(name="state", bufs=1) as st,
            tc.tile_pool(name="scr", bufs=3) as sc,
            tc.tile_pool(name="pool_scr", bufs=2) as pscr,
            tc.tile_pool(name="psum", bufs=3, space="PSUM") as pp,
            tc.tile_pool(name="psum_w", bufs=1, space="PSUM") as ppw,
        ):
            wsb = st.tile([128, 9, 256], BF16, tag="wsb")
            nc.sync.dma_start(out=wsb, in_=w_d[:, :, :])
            cons = st.tile([128, 4], F32, tag="cons")
            nc.sync.dma_start(out=cons, in_=cn_d[:, :])
            b_fi = cons[:, 0:1]       # 0.2*b_f + 0.5 ; 0.2*b_i + 0.5
            b_o = cons[0:64, 1:2]     # 0.5 - 0.2*b_o
            b_g = cons[64:128, 1:2]   # b_g
            bns = cons[0:64, 2:3]     # BN scale
            bnb = cons[0:64, 3:4]     # BN bias

            # planes: partitions 0:63 = h_t, 64:127 = x_t, double buffered
            xh = [st.tile([128, RP * CP], BF16, tag=f"xh{i}", name=f"xh{i}")
                  for i in range(2)]
            cg = st.tile([128, HS * W], BF16, tag="cg")
            for tns in xh:
                nc.sync.dma_start(out=tns, in_=z_d[:, :])
            nc.vector.memset(cg, 0.0)

            # ramp the PE p-state while the first x tile is in flight
            wflat = wsb.rearrange("p a b -> p (a b)")
            wps = ppw.tile([128, 512], F32, tag="warm")
            for _ in range(8):
                nc.tensor.matmul(wps, wsb[:, 0, 0:128], wflat[:, 0:512],
                                 start=True, stop=True)

            def pv(tns):
                return tns.rearrange("p (r c) -> p r c", r=RP)

            nc.sync.dma_start(out=pv(xh[0])[64:128, 1:49, 1:65],
                              in_=x_d[0, :, :, :])

            for t in range(T):
                cur = pv(xh[t % 2])
                nxt = pv(xh[(t + 1) % 2])
                if t + 1 < T:
                    nc.sync.dma_start(out=nxt[64:128, 1:49, 1:65],
                                      in_=x_d[t + 1, :, :, :])

                for blk in range(_nblk(t)):
                    r0 = blk * 8
                    fs = slice(blk * 512, (blk + 1) * 512)
                    pst = []
                    for mh in range(2):
                        ps = pp.tile([128, 512], F32, tag=f"ps{mh}",
                                     name=f"ps_{t}_{blk}_{mh}")
                        pst.append(ps)
                        for j in range(9):
                            a0, b0 = j // 3, j % 3
                            rhs = cur[:, r0 + a0:r0 + a0 + 8, b0:b0 + 64]
                            nc.tensor.matmul(
                                ps, wsb[:, j, mh * 128:(mh + 1) * 128], rhs,
                                start=(j == 0), stop=(j == 8))
                    ps0, ps1 = pst

                    # f,i: clip(z' + b', 0, 1) on DVE (scale folded in W)
                    fi2 = sc.tile([128, 512], BF16, tag="fi2")
                    nc.vector.tensor_scalar(fi2, ps0, b_fi, 0.0,
                                            ALU.add, ALU.max)
                    nc.vector.tensor_scalar_min(fi2, fi2, 1.0)
                    # g first - it gates the h critical path
                    nc.scalar.activation(cg[64:128, fs], ps1[64:128, :],
                                         ACTF.Tanh, bias=b_g, scale=1.0)
                    # o: hard_sigmoid via Relu chain on ACT
                    # (o columns folded with -0.2): relu(1 - relu(z''+b''))
                    oo1 = sc.tile([64, 512], BF16, tag="oo1")
                    nc.scalar.activation(oo1, ps1[0:64, :], ACTF.Relu,
                                         bias=b_o, scale=1.0)
                    oo = sc.tile([64, 512], BF16, tag="oo")
                    nc.scalar.activation(oo, oo1, ACTF.Relu,
                                         bias=1.0, scale=-1.0)
                    # pr = [f*c ; i*g]; fold i*g down via the Pool DGE
                    pr = sc.tile([128, 512], BF16, tag="pr")
                    nc.vector.tensor_tensor(pr, fi2, cg[:, fs], ALU.mult)
                    prm = sc.tile([64, 512], BF16, tag="prm")
                    nc.gpsimd.dma_start(out=prm, in_=pr[64:128, :])
                    nc.vector.tensor_tensor(cg[0:64, fs], pr[0:64, :], prm,
                                            ALU.add)
                    # h = o * tanh(c), written straight into next plane
                    tct = sc.tile([64, 512], BF16, tag="tct")
                    nc.scalar.activation(tct, cg[0:64, fs], ACTF.Tanh)
                    nc.vector.tensor_tensor(
                        nxt[0:64, r0 + 1:r0 + 9, 1:65],
                        tct.rearrange("p (r c) -> p r c", r=8),
                        oo.rearrange("p (r c) -> p r c", r=8), ALU.mult)

                # BN + 2x2 max pool on h_{t+1}, owned rows 0:32 only
                s1 = pscr.tile([64, 32, 32], BF16, tag="s1")
                nc.vector.tensor_tensor(s1, nxt[0:64, 1:33, 1:65:2],
                                        nxt[0:64, 1:33, 2:66:2], ALU.max)
                s2 = pscr.tile([64, 16, 32], BF16, tag="s2")
                nc.vector.tensor_tensor(s2, s1[:, 0:32:2, :], s1[:, 1:32:2, :],
                                        ALU.max)
                yt = pscr.tile([64, 16 * 32], F32, tag="yt")
                nc.scalar.activation(yt, s2.rearrange("p a b -> p (a b)"),
                                     ACTF.Identity, bias=bnb, scale=bns)
                nc.sync.dma_start(out=y_d[t, :, :], in_=yt)

    _split_multi_waits(nc)
    return nc


def _get_nc():
    global _cached
    if _cached is None:
        _cached = _build()
    return _cached


def _prep_inputs(input_tensor, W_, U, b, gamma, beta, moving_mean, moving_var):
    x = np.asarray(input_tensor, np.float32)
    W_ = np.asarray(W_, np.float32)
    U = np.asarray(U, np.float32)
    b = np.asarray(b, np.float32)
    gamma = np.asarray(gamma, np.float32)
    beta = np.asarray(beta, np.float32)
    moving_mean = np.asarray(moving_mean, np.float32)
    moving_var = np.asarray(moving_var, np.float32)

    # Cout reorder (i,f,g,o) -> (f,i,o,g); fold hard_sigmoid scales into
    # the f,i (+0.2) and o (-0.2) columns.
    perm = [1, 0, 3, 2]
    gs = np.array([0.2, 0.2, -0.2, 1.0], np.float32)[None, None, None, :, None]
    Wr = (W_.reshape(3, 3, C, 4, C)[:, :, :, perm, :] * gs).reshape(3, 3, C, 4 * C)
    Ur = (U.reshape(3, 3, C, 4, C)[:, :, :, perm, :] * gs).reshape(3, 3, C, 4 * C)
    # stationary tap j: rows 0:64 = U tap (h half), rows 64:128 = W tap (x);
    # wstk[1] has the kernel rows flipped, for the row-flipped bottom shards
    wstk = np.zeros((2, 9, 128, 256), np.float32)
    for j in range(9):
        a0, b0 = j // 3, j % 3
        wstk[0, j, 0:64] = Ur[a0, b0]
        wstk[0, j, 64:128] = Wr[a0, b0]
        wstk[1, j, 0:64] = Ur[2 - a0, b0]
        wstk[1, j, 64:128] = Wr[2 - a0, b0]
    wstk = np.ascontiguousarray(wstk.transpose(0, 2, 1, 3)).astype(
        ml_dtypes.bfloat16)  # [2, 128, 9, 256]

    b4 = b.reshape(4, C)[perm]  # rows f,i,o,g
    consts = np.zeros((128, 4), np.float32)
    consts[0:64, 0] = 0.2 * b4[0] + 0.5
    consts[64:128, 0] = 0.2 * b4[1] + 0.5
    consts[0:64, 1] = 0.5 - 0.2 * b4[2]
    consts[64:128, 1] = b4[3]
    scale = gamma / np.sqrt(moving_var + BN_EPS)
    consts[0:64, 2] = scale
    consts[0:64, 3] = beta - moving_mean * scale

    zplane = np.zeros((128, RP * CP), ml_dtypes.bfloat16)
    in_maps = []
    for k in range(8):
        s, half = k // 2, k % 2
        if half == 0:
            xs = x[s, :, 0:HS]
        else:
            xs = x[s, :, H - HS:H][:, ::-1]  # row-flipped bottom shard
        xs = np.ascontiguousarray(xs.transpose(0, 3, 1, 2)).astype(
            ml_dtypes.bfloat16)
        in_maps.append({"xc": xs, "wstk": wstk[half], "consts": consts,
                        "zeros": zplane})
    return in_maps


def kernel(input_tensor, W, U, b, gamma, beta, moving_mean, moving_var):
    in_maps = _prep_inputs(input_tensor, W, U, b, gamma, beta,
                           moving_mean, moving_var)
    res = run_bass_kernel_spmd(_get_nc(), in_maps, core_ids=list(range(8)))

    out = np.empty((B, T, 32, 32, C), np.float32)
    for k in range(8):
        s, half = k // 2, k % 2
        yc = res.results[k]["yout"].reshape(T, C, 16, 32).transpose(0, 2, 3, 1)
        if half == 0:
            out[s, :, 0:16] = yc
        else:
            out[s, :, 16:32] = yc[:, ::-1]
    return out
